# revision 1
# baseline (speedup 1.0000x reference)
"""Trainium2 Bass kernel for a CapsNet dynamic-routing layer.

Math (per batch b):
    u_hat[n, m] = u_vecs[b] @ kernel[0]          # [2048,64] @ [64,512]
    u_hat grouped as 32 capsules x 16 dims: m = i*16 + j
    3 rounds of routing:
        c = softmax_i(b_logits)                   # uniform on round 0
        o[i, j] = sum_n c[n, i] * u_hat[n, (i,j)]
        (rounds 0,1) o_n = o / ||o_i||_2 ;  b_logits[n, i] = <o_n[i,:], u_hat[., (i,.)]>
    out = squash(o)

Distribution: data-parallel over batch. 64 batches -> 8 NeuronCores x 8 batches.
The routing loop is fully batch-local; the only shared tensor (kernel, 64x512)
is replicated, so there are no collectives.

Per-core layout strategy (all fp32):
  - uT  [64, 2048]   : u[b].T, built by PE transposes (d on partitions)
  - A   [128,16,512] : u_hat with n on partitions (16 chunks of 128 n)
                       -> moving operand of the output contraction
  - B   [128,4,2048] : u_hat.T with m=(i,j) on partitions (4 tiles of 8 caps)
                       -> moving operand of the agreement contraction
  - softmax runs in [n-partition, (chunk, i)-free] layout obtained by
    compacting the agreement output with a constant selection matrix on PE.
"""

from contextlib import ExitStack

import numpy as np

import concourse.bacc as bacc
import concourse.bass as bass
import concourse.tile as tile
from concourse import mybir
from concourse.bass_utils import run_bass_kernel_spmd

F32 = mybir.dt.float32
F32R = mybir.dt.float32r   # PE single-pass fp32 (~1e-4 rel, 4x faster, N>=256)
AF = mybir.ActivationFunctionType
ALU = mybir.AluOpType

USE_F32R = True
MMDT = F32R if USE_F32R else F32

# Force Exp and Ln activations to resolve to the one table set that holds both
# ("natural_log_exp_and_others"): otherwise the table-load pass alternates
# exp<->ln set loads (~2.7us each) every routing iteration. Indices into
# act_info.json are preserved; only the per-set function contents shrink.
_orig_get_activation_tables = bacc.get_activation_tables


def _patched_get_activation_tables(module_arch):
    tabs = _orig_get_activation_tables(module_arch)
    target = "natural_log_exp_and_others"
    if target in tabs and {AF.Exp, AF.Ln} <= tabs[target]:
        for name, funcs in tabs.items():
            if name != target:
                funcs.discard(AF.Exp)
                funcs.discard(AF.Ln)
    return tabs


bacc.get_activation_tables = _patched_get_activation_tables

# Problem constants (hardcoded per contest contract)
B_FULL = 64
N_CORES = 8
B_LOC = B_FULL // N_CORES      # 8 batches per core
N_IN = 2048                    # input capsules
D_IN = 64                      # input dim
NUM_CAP = 32
DIM_CAP = 16
M = NUM_CAP * DIM_CAP          # 512
NCHUNK = N_IN // 128           # 16 chunks of n
ROUTINGS = 3
EPS = 1e-7
L2_EPS = 1e-12

_cached = {}


def build_bass(repeat: int = 1):
    nc = bacc.Bacc("TRN2", target_bir_lowering=False, debug=False)

    u_d = nc.declare_dram_parameter("u", [B_LOC, N_IN, D_IN], F32, isOutput=False)
    w_d = nc.declare_dram_parameter("w", [D_IN, M], F32, isOutput=False)
    out_d = nc.declare_dram_parameter("out", [B_LOC, NUM_CAP, DIM_CAP], F32, isOutput=True)

    u_ap = u_d.ap()
    w_ap = w_d.ap()
    out_ap = out_d.ap()

    with tile.TileContext(nc) as tc, ExitStack() as ctx:
        consts = ctx.enter_context(tc.tile_pool(name="consts", bufs=1))
        u_pool = ctx.enter_context(tc.tile_pool(name="u_pool", bufs=2))
        ut_pool = ctx.enter_context(tc.tile_pool(name="ut_pool", bufs=2))
        a_pool = ctx.enter_context(tc.tile_pool(name="a_pool", bufs=2))
        b_pool = ctx.enter_context(tc.tile_pool(name="b_pool", bufs=2))
        r_pool = ctx.enter_context(tc.tile_pool(name="r_pool", bufs=2))   # routing small tiles
        bl_pool = ctx.enter_context(tc.tile_pool(name="bl_pool", bufs=1))  # b logits sbuf
        ps_main = ctx.enter_context(tc.tile_pool(name="ps_main", bufs=2, space="PSUM"))
        ps_small = ctx.enter_context(tc.tile_pool(name="ps_small", bufs=2, space="PSUM"))
        ps_b = ctx.enter_context(tc.tile_pool(name="ps_b", bufs=2, space="PSUM"))
        ps_opool = ctx.enter_context(tc.tile_pool(name="ps_opool", bufs=2, space="PSUM"))

        # ---------------- constants ----------------
        # kick off the first input DMA before anything else
        u_first = u_pool.tile([128, NCHUNK, D_IN], F32, tag="u_nat")
        nc.sync.dma_start(
            out=u_first,
            in_=u_ap[0].rearrange("(c p) d -> p c d", p=128),
        )
        # W duplicated into both partition halves: rows 0-63 and 64-127
        w_sb = consts.tile([128, M], F32)
        nc.sync.dma_start(out=w_sb[0:64, :], in_=w_ap)
        nc.sync.dma_start(out=w_sb[64:128, :], in_=w_ap)

        w_r = consts.tile([128, M], MMDT)
        nc.vector.tensor_copy(w_r[0:64, :], w_sb[0:64, :])
        nc.vector.tensor_copy(w_r[64:128, :], w_sb[64:128, :])

        ones128 = consts.tile([128, 128], F32)
        nc.vector.memset(ones128, 1.0)

        # I128 identity for PE transposes
        i128 = consts.tile([128, 128], F32)
        nc.gpsimd.affine_select(
            out=i128, in_=ones128, pattern=[[1, 128]],
            compare_op=ALU.is_equal, fill=0.0, base=0, channel_multiplier=-1,
        )
        # I32 identity (for diag(rn))
        i32 = consts.tile([32, 32], F32)
        nc.gpsimd.affine_select(
            out=i32, in_=ones128[0:32, 0:32], pattern=[[1, 32]],
            compare_op=ALU.is_equal, fill=0.0, base=0, channel_multiplier=-1,
        )
        # c0: uniform softmax output 1/32 (memset can't write f32r: round-copy)
        c0_f = consts.tile([128, NUM_CAP], F32)
        nc.vector.memset(c0_f, 1.0 / NUM_CAP)
        c0 = consts.tile([128, NUM_CAP], MMDT)
        nc.vector.tensor_copy(c0, c0_f)

        # dmask [32, 512]: dmask[i, m] = 1 if m//16 == i else 0
        dmask = consts.tile([NUM_CAP, M], F32)
        dm_tmp = consts.tile([NUM_CAP, M], F32)
        ones32x512 = consts.tile([NUM_CAP, M], F32)
        nc.vector.memset(ones32x512, 1.0)
        nc.gpsimd.affine_select(
            out=dm_tmp, in_=ones32x512, pattern=[[1, M]],
            compare_op=ALU.is_ge, fill=0.0, base=0, channel_multiplier=-DIM_CAP,
        )
        # keep where 15 - (m - 16 i) >= 0  (is_le unimplemented in walrus)
        nc.gpsimd.affine_select(
            out=dmask, in_=dm_tmp, pattern=[[-1, M]],
            compare_op=ALU.is_ge, fill=0.0, base=DIM_CAP - 1,
            channel_multiplier=DIM_CAP,
        )
        # E_sel [128, 32]: 4 stacked 32x32 identities. The agreement matmul for
        # strip t produces cap i at psum partition 32t+i (nonzero only for
        # i in [8t, 8t+8)); summing b_sb[32t+i, :] over t via E recovers cap i.
        e_sel = consts.tile([128, NUM_CAP], F32)
        for t in range(4):
            nc.gpsimd.affine_select(
                out=e_sel[32 * t:32 * t + 32, :], in_=ones128[0:32, 0:NUM_CAP],
                pattern=[[1, NUM_CAP]],
                compare_op=ALU.is_equal, fill=0.0, base=0,
                channel_multiplier=-1,
            )

        # ic4 [32, 512]: ones at (i, 128t + 32t + i) — per t-block an identity
        # at sub-block q=t. Used to build the block-padded Wo stationary.
        ic4 = consts.tile([NUM_CAP, M], F32)
        nc.vector.memset(ic4, 0.0)
        for t in range(4):
            nc.gpsimd.affine_select(
                out=ic4[:, 160 * t:160 * t + NUM_CAP],
                in_=ones128[0:NUM_CAP, 0:NUM_CAP],
                pattern=[[1, NUM_CAP]],
                compare_op=ALU.is_equal, fill=0.0, base=0,
                channel_multiplier=-1,
            )

        # eps tiles used as activation bias (const DB only has 0.0/1.0)
        eps12 = consts.tile([128, 1], F32)
        nc.vector.memset(eps12, L2_EPS)
        eps7 = consts.tile([128, 1], F32)
        nc.vector.memset(eps7, EPS)

        # ---------------- phase helpers (1-batch software pipeline) ----------
        def load_u(b):
            u_nat = u_pool.tile([128, NCHUNK, D_IN], F32, name="u_nat", tag="u_nat")
            nc.sync.dma_start(
                out=u_nat,
                in_=u_ap[b].rearrange("(c p) d -> p c d", p=128),
            )
            return u_nat

        def transpose_u_thunks(u_nat, ut_sb):
            """u [128, c, 64] -> uT [64, 2048]; one thunk per transpose, the
            4th of each group also emits the psum->sbuf copy."""
            state = {}

            def mk(g, k):
                def emit():
                    if k == 0:
                        state[g] = ps_main.tile(
                            [64, 512], F32, name="ps_ut", tag="ps_main"
                        )
                    ps_ut = state[g]
                    c_ = 4 * g + k
                    nc.tensor.transpose(
                        out=ps_ut[:, 128 * k:128 * k + 128],
                        in_=u_nat[:, c_, :],
                        identity=i128,
                    )
                    if k == 3:
                        nc.scalar.copy(
                            ut_sb[:, 512 * g:512 * g + 512], ps_ut
                        )
                return emit
            return [mk(g, k) for g in range(4) for k in range(4)]

        def transpose_u(u_nat):
            ut_sb = ut_pool.tile([64, N_IN], MMDT, tag="ut_sb")
            for th in transpose_u_thunks(u_nat, ut_sb):
                th()
            return ut_sb

        def build_a_thunks(ut_sb, a_sb):
            """u_hat, n on partitions: A[p, c, m] — one thunk per chunk."""
            def mk(c_):
                def emit():
                    ps = ps_main.tile([128, M], F32, tag="ps_main")
                    nc.tensor.matmul(
                        ps,
                        lhsT=ut_sb[:, 128 * c_:128 * c_ + 128],
                        rhs=w_r[0:64, :],
                        start=True, stop=True,
                    )
                    if c_ % 2 == 0:
                        nc.scalar.copy(a_sb[:, c_, :], ps)
                    else:
                        nc.vector.tensor_copy(a_sb[:, c_, :], ps)
                return emit
            return [mk(c_) for c_ in range(NCHUNK)]

        def build_b_thunks(ut_sb, b_sb):
            """u_hat.T, m on partitions: B[p, t, n] — one thunk per (t, g)."""
            def mk(t, g):
                def emit():
                    ps = ps_main.tile([128, M], F32, tag="ps_main")
                    nc.tensor.matmul(
                        ps,
                        lhsT=w_r[0:64, 128 * t:128 * t + 128],
                        rhs=ut_sb[:, 512 * g:512 * g + 512],
                        start=True, stop=True,
                    )
                    if (t + g) % 2 == 0:
                        nc.scalar.copy(b_sb[:, t, 512 * g:512 * g + 512], ps)
                    else:
                        nc.vector.tensor_copy(b_sb[:, t, 512 * g:512 * g + 512], ps)
                return emit
            return [mk(t, g) for g in range(4) for t in range(4)]

        def out_contract(a_sb, c_sb):
            """o_full[i, m] = sum_n c[n,i] u_hat[n,m] -> psum [32, 512]."""
            ps_o = ps_opool.tile([NUM_CAP, M], F32, tag="ps_o")
            for c_ in range(NCHUNK):
                lhs = c0 if c_sb is None else c_sb[:, c_, :]
                nc.tensor.matmul(
                    ps_o,
                    lhsT=lhs,
                    rhs=a_sb[:, c_, :],
                    start=(c_ == 0), stop=(c_ == NCHUNK - 1),
                )
            return ps_o

        def norm_glue(ps_o):
            """om (masked o), rn = 1/sqrt(sum o^2 + eps), dmat = diag(rn)."""
            om = r_pool.tile([NUM_CAP, M], F32, tag="om")
            nc.vector.tensor_mul(om, ps_o, dmask)
            sq = r_pool.tile([NUM_CAP, M], F32, tag="sq")
            s = r_pool.tile([NUM_CAP, 1], F32, tag="s")
            nc.vector.tensor_mul(sq, om, om)
            nc.vector.reduce_sum(s, sq, axis=mybir.AxisListType.X)
            lns = r_pool.tile([NUM_CAP, 1], F32, tag="lns")
            nc.scalar.activation(lns, s, AF.Ln, bias=eps12[0:NUM_CAP])
            rn = r_pool.tile([NUM_CAP, 1], F32, tag="rn")
            nc.scalar.activation(rn, lns, AF.Exp, scale=-0.5)
            dp = r_pool.tile([NUM_CAP, M], F32, tag="dp")
            nc.vector.tensor_scalar_mul(dp, ic4, rn)
            return om, dp

        def ot_wo(om, dp):
            """WoPad_t[p=(l,j), 32q+i] = o_norm[i, j]*d(q==t) on diag strips.

            Block t's stationary is zero outside col-block q=t, so the four
            agreement matmuls can accumulate into one base-0 psum (f32r
            matmuls require dst partition base 0).
            """
            ps_ot = ps_small.tile([128, 4, 128], F32, tag="ps_small")
            for t in range(4):
                nc.tensor.matmul(
                    ps_ot[:, t, :],
                    lhsT=om[:, 128 * t:128 * t + 128],
                    rhs=dp[:, 128 * t:128 * t + 128],
                    start=True, stop=True,
                )
            wo = r_pool.tile([128, 4, 128], MMDT, tag="wo")
            nc.scalar.copy(wo, ps_ot)
            return wo

        def fused_asco(wo, b_sb, a_sb, fill):
            """Streamed agree -> compact -> softmax -> next output contraction.

            Per n-slice g (512 n): 4 agreement MMs, psum->sbuf copy, 4 compact
            MMs, exp/sum/recip/mul softmax on that slice, then the 4 output-
            contraction MMs for those chunks accumulate into the next o psum.
            Stages of different g overlap across engines.
            """
            blog = bl_pool.tile([128, 4, 512], F32)
            ps_bt = ps_small.tile([128, NCHUNK, NUM_CAP], F32, tag="ps_small")
            e_sb = r_pool.tile([128, NCHUNK, NUM_CAP], F32, tag="e_sb")
            den = r_pool.tile([128, NCHUNK], F32, tag="den")
            rden = r_pool.tile([128, NCHUNK], F32, tag="rden")
            c_sb = r_pool.tile([128, NCHUNK, NUM_CAP], MMDT, tag="c_sb")
            ps_o = ps_opool.tile([NUM_CAP, M], F32, tag="ps_o")
            for g in range(4):
                # accumulate the 4 disjoint-row strip blocks into one
                # base-0 psum (f32r dst constraint)
                ps_bl = ps_b.tile([128, 512], F32, tag="ps_b")
                for t in range(4):
                    nc.tensor.matmul(
                        ps_bl,
                        lhsT=wo[:, t, :],
                        rhs=b_sb[:, t, 512 * g:512 * g + 512],
                        start=(t == 0), stop=(t == 3),
                    )
                    fill(1)
                if g % 2 == 0:
                    nc.scalar.copy(blog[:, g, :], ps_bl)
                else:
                    nc.vector.tensor_copy(blog[:, g, :], ps_bl)
                for k in range(4):
                    c_ = 4 * g + k
                    nc.tensor.matmul(
                        ps_bt[:, c_, :],
                        lhsT=blog[:, g, 128 * k:128 * k + 128],
                        rhs=e_sel,
                        start=True, stop=True,
                    )
                    fill(1)
                sl = slice(4 * g, 4 * g + 4)
                nc.scalar.activation(e_sb[:, sl, :], ps_bt[:, sl, :], AF.Exp)
                nc.vector.reduce_sum(
                    den[:, sl], e_sb[:, sl, :], axis=mybir.AxisListType.X
                )
                nc.vector.reciprocal(rden[:, sl], den[:, sl])
                nc.vector.tensor_mul(
                    c_sb[:, sl, :], e_sb[:, sl, :],
                    rden[:, sl].unsqueeze(-1).broadcast_to((128, 4, NUM_CAP)),
                )
                for k in range(4):
                    c_ = 4 * g + k
                    nc.tensor.matmul(
                        ps_o,
                        lhsT=c_sb[:, c_, :],
                        rhs=a_sb[:, c_, :],
                        start=(c_ == 0), stop=(c_ == NCHUNK - 1),
                        skip_group_check=True,
                    )
                    fill(1)
            return ps_o

        def squash_store(ps_o, b):
            om = r_pool.tile([NUM_CAP, M], F32, tag="om")
            nc.vector.tensor_mul(om, ps_o, dmask)
            oc = r_pool.tile([NUM_CAP, DIM_CAP], F32, tag="oc")
            om_v = om.rearrange("p (i j) -> p j i", j=DIM_CAP)
            nc.vector.reduce_sum(oc, om_v, axis=mybir.AxisListType.X)
            sq2 = r_pool.tile([NUM_CAP, DIM_CAP], F32, tag="sq2")
            s2 = r_pool.tile([NUM_CAP, 1], F32, tag="s2")
            nc.vector.tensor_mul(sq2, oc, oc)
            nc.vector.reduce_sum(s2, sq2, axis=mybir.AxisListType.X)
            ln2 = r_pool.tile([NUM_CAP, 1], F32, tag="ln2")
            nc.scalar.activation(ln2, s2, AF.Ln, bias=eps7[0:NUM_CAP])
            rt2 = r_pool.tile([NUM_CAP, 1], F32, tag="rt2")
            nc.scalar.activation(rt2, ln2, AF.Exp, scale=0.5)  # sqrt(s2+eps)
            den2 = r_pool.tile([NUM_CAP, 1], F32, tag="den2")
            nc.vector.tensor_scalar_add(den2, s2, 0.5 + EPS)
            rden2 = r_pool.tile([NUM_CAP, 1], F32, tag="rden2")
            nc.vector.reciprocal(rden2, den2)
            scl = r_pool.tile([NUM_CAP, 1], F32, tag="scl")
            nc.vector.tensor_mul(scl, rt2, rden2)
            ov = r_pool.tile([NUM_CAP, DIM_CAP], F32, tag="ov")
            nc.vector.tensor_scalar_mul(ov, oc, scl)
            nc.sync.dma_start(out=out_ap[b], in_=ov)

        # optional repeat loop for wall-clock benchmarking (repeat > 1)
        rep_cm = tc.For_i(0, repeat, 1) if repeat > 1 else None
        if rep_cm is not None:
            rep_cm.__enter__()

        # ---------------- pipelined batch loop ----------------
        # Interleave batch b's routing with batch b+1's u_hat builds: build
        # matmuls are emitted one-at-a-time between routing matmuls so the PE
        # FIFO never stalls long on a psum-slot copy.
        ut = transpose_u(u_first)
        a_cur = a_pool.tile([128, NCHUNK, M], MMDT, tag="a_sb")
        b_cur = b_pool.tile([128, 4, N_IN], MMDT, tag="b_sb")
        for th in build_a_thunks(ut, a_cur) + build_b_thunks(ut, b_cur):
            th()

        for b in range(B_LOC):
            have_next = b + 1 < B_LOC
            pending = []
            if have_next:
                u_nxt = load_u(b + 1)
                a_nxt = a_pool.tile([128, NCHUNK, M], MMDT, tag="a_sb")
                b_nxt = b_pool.tile([128, 4, N_IN], MMDT, tag="b_sb")
                ut_nxt = ut_pool.tile([64, N_IN], MMDT, tag="ut_sb")
                pending = (
                    transpose_u_thunks(u_nxt, ut_nxt)
                    + build_a_thunks(ut_nxt, a_nxt)
                    + build_b_thunks(ut_nxt, b_nxt)
                )

            filler = iter(pending)
            paced = [0]

            def fill_now(n, _f=filler):
                for _ in range(n):
                    th = next(_f, None)
                    if th is None:
                        return
                    th()

            def fill(n, _f=filler, _p=paced):
                # one thunk per three requested slots (fused has ~96 slots,
                # the norm-glue gaps take the rest unpaced)
                for _ in range(n):
                    _p[0] += 1
                    if _p[0] % 3 == 0:
                        th = next(_f, None)
                        if th is None:
                            return
                        th()

            def fill_rest(_f=filler):
                for th in _f:
                    th()

            # --- routing (iter 0 output uses uniform c) ---
            ps_o = out_contract(a_cur, None)
            for _r in range(ROUTINGS - 1):
                fill_now(3)
                om, dp = norm_glue(ps_o)
                fill_now(3)
                wo = ot_wo(om, dp)
                fill_now(2)
                ps_o = fused_asco(wo, b_cur, a_cur, fill)
            fill_rest()
            squash_store(ps_o, b)

            if have_next:
                a_cur, b_cur = a_nxt, b_nxt

        if rep_cm is not None:
            rep_cm.__exit__(None, None, None)

    nc.compile()
    return nc


def kernel(u_vecs: np.ndarray, kernel: np.ndarray) -> np.ndarray:
    assert u_vecs.shape == (B_FULL, N_IN, D_IN)
    w = np.ascontiguousarray(kernel.reshape(D_IN, M), dtype=np.float32)
    u_vecs = np.ascontiguousarray(u_vecs, dtype=np.float32)

    if "nc" not in _cached:
        _cached["nc"] = build_bass()
    nc = _cached["nc"]

    in_maps = []
    for core in range(N_CORES):
        shard = u_vecs[core * B_LOC:(core + 1) * B_LOC]
        in_maps.append({"u": np.ascontiguousarray(shard), "w": w})

    res = run_bass_kernel_spmd(nc, in_maps, core_ids=list(range(N_CORES)))
    outs = [res.results[c]["out"] for c in range(N_CORES)]
    return np.concatenate(outs, axis=0)

